# revision 10
# baseline (speedup 1.0000x reference)
"""Int8RouterLinear TRN2 kernel: out[16384, 64] = x[16384, 4096] @ (W_int8 * scale)^T.

Strategy (data-parallel over 8 NeuronCores, 2048 tokens each):
  - x (f32) is viewed as pairs of u16; the hi-u16 of each f32 IS bf16(x)
    (truncated). The XBAR DMA-transpose (2-byte dtypes, DRAM->SBUF) loads
    x^T directly into SBUF: xt[p, g, t] = x_u16[t, g*128 + p]. Odd
    partitions hold bf16(x) for h = 64g + (p-1)/2; even partitions hold
    the lo-u16 garbage (can be NaN-patterned).
  - A DVE copy_predicated pass overwrites even (garbage) partitions with
    1.0 (mask doubles as data); the stationary weights have 0.0 there, so
    garbage contributes exactly 0 to the contraction.
  - The router weight is tiny: host dequantizes + lays it out as
    w[p, g, e] = bf16(W^T[64g + (p-1)/2, e]) on odd p, 0.0 on even p.
  - PE then runs ONLY regular bf16 matmuls (64 accumulating MMs per
    256-token slab, moving N=256) - no PE transposes, no PSUM round-trip
    for x, and the dense MM stream keeps the HAM clock-gate warm.
  - All XBAR transposes are serialized on the sync queue (concurrent
    XBAR transposes hard-wedge the device); output DMAs interleave on the
    same queue two slabs behind, so nothing ever runs beside the XBAR.
  - out^T chunks DMA to DRAM as [64, 2048]; host transposes/concats (4MB).
"""
import numpy as np
import ml_dtypes

import concourse.mybir as mybir
from concourse import bacc
from concourse.tile import TileContext
from concourse.bass_utils import run_bass_kernel_spmd

TOKENS = 16384
HIDDEN = 4096
EXPERTS = 64
NCORES = 8
TSHARD = TOKENS // NCORES          # 2048 tokens per core
C = 2 * HIDDEN                     # 8192 u16 columns
G = C // 128                       # 64 column-groups per transpose
SLAB = 256                         # tokens per XBAR transpose
NSLAB = TSHARD // SLAB             # 8
FIXG = 8                           # g-groups per copy_predicated call

BF16 = mybir.dt.bfloat16
F32 = mybir.dt.float32

_cache = {}


def _build():
    if "nc" in _cache:
        return _cache["nc"]
    nc = bacc.Bacc("TRN2", target_bir_lowering=False, debug=False,
                   num_devices=NCORES)
    x_d = nc.dram_tensor("x", [TSHARD, C], BF16, kind="ExternalInput")
    w_d = nc.dram_tensor("w", [128, G * EXPERTS], BF16, kind="ExternalInput")
    m_d = nc.dram_tensor("m", [128, FIXG * SLAB], mybir.dt.uint16,
                         kind="ExternalInput")
    o_d = nc.dram_tensor("out", [EXPERTS, TSHARD], F32, kind="ExternalOutput")
    x = x_d.ap()
    o = o_d.ap()

    with TileContext(nc) as tc:
        with tc.tile_pool(name="consts", bufs=1) as cpool, \
             tc.tile_pool(name="xt", bufs=3) as xtpool, \
             tc.tile_pool(name="ps", bufs=2, space="PSUM") as pspool, \
             tc.tile_pool(name="ot", bufs=2) as otpool:
            w_sb = cpool.tile([128, G, EXPERTS], BF16)
            nc.sync.dma_start(out=w_sb, in_=w_d.ap())
            mask = cpool.tile([128, FIXG, SLAB], mybir.dt.uint16)
            nc.sync.dma_start(out=mask, in_=m_d.ap())

            outs = []

            def drain_out(upto):
                while len(outs) > upto:
                    s0, ot0 = outs.pop(0)
                    nc.sync.dma_start(
                        out=o[:, s0 * SLAB:(s0 + 1) * SLAB], in_=ot0)

            for s in range(NSLAB):
                xt = xtpool.tile([128, G, SLAB], BF16)
                nc.sync.dma_start_transpose(
                    xt, x[s * SLAB:(s + 1) * SLAB, :])
                # keep out-DMAs 2 slabs behind so they never stall the queue
                drain_out(2)

                po = pspool.tile([EXPERTS, SLAB], F32)
                for j in range(G // FIXG):
                    xv = xt[:, j * FIXG:(j + 1) * FIXG, :].bitcast(
                        mybir.dt.uint16)
                    nc.vector.tensor_tensor(
                        out=xv, in0=xv, in1=mask,
                        op=mybir.AluOpType.bitwise_and)
                    for g in range(j * FIXG, (j + 1) * FIXG):
                        nc.tensor.matmul(po, w_sb[:, g, :], xt[:, g, :],
                                         start=(g == 0), stop=(g == G - 1))
                ot = otpool.tile([EXPERTS, SLAB], F32)
                nc.vector.tensor_copy(ot, po)
                outs.append((s, ot))
            drain_out(0)

    nc.compile()
    _cache["nc"] = nc
    return nc


def _prep_w(weights_int8, scales):
    wt = (weights_int8.astype(np.float32)
          * scales.astype(np.float32)[:, None]).T   # [HIDDEN, EXPERTS]
    warr = np.zeros((128, G, EXPERTS), dtype=ml_dtypes.bfloat16)
    warr[1::2, :, :] = wt.reshape(G, 64, EXPERTS).transpose(1, 0, 2)
    return np.ascontiguousarray(warr.reshape(128, G * EXPERTS))


def _prep_mask():
    m = np.zeros((128, FIXG * SLAB), dtype=np.uint16)
    m[1::2, :] = 0xFFFF          # odd partitions hold real data - keep
    return m


def _make_inmaps(x, weights_int8, scales):
    warr = _prep_w(weights_int8, scales)
    marr = _prep_mask()
    xu = np.ascontiguousarray(x, dtype=np.float32).view(np.uint16)
    xv = xu.reshape(TOKENS, C).view(ml_dtypes.bfloat16)
    return [
        {"x": xv[c * TSHARD:(c + 1) * TSHARD], "w": warr, "m": marr}
        for c in range(NCORES)
    ]


def kernel(x, weights_int8, scales):
    nc = _build()
    in_maps = _make_inmaps(x, weights_int8, scales)
    res = run_bass_kernel_spmd(nc, in_maps, core_ids=list(range(NCORES)))
    out = np.concatenate(
        [np.asarray(res.results[c]["out"]).T for c in range(NCORES)], axis=0)
    return np.ascontiguousarray(out, dtype=np.float32)


# revision 11
# speedup vs baseline: 1.0659x; 1.0659x over previous
"""Int8RouterLinear TRN2 kernel: out[16384, 64] = x[16384, 4096] @ (W_int8 * scale)^T.

Strategy (data-parallel over 8 NeuronCores, 2048 tokens each):
  - x (f32) is viewed as pairs of u16; the hi-u16 of each f32 IS bf16(x)
    (truncated). The XBAR DMA-transpose (2-byte dtypes, DRAM->SBUF) loads
    x^T directly into SBUF: xt[p, g, t] = x_u16[t, g*128 + p]. Odd
    partitions hold bf16(x) for h = 64g + (p-1)/2; even partitions hold
    the lo-u16 garbage (can be NaN-patterned).
  - A DVE copy_predicated pass overwrites even (garbage) partitions with
    1.0 (mask doubles as data); the stationary weights have 0.0 there, so
    garbage contributes exactly 0 to the contraction.
  - The router weight is tiny: host dequantizes + lays it out as
    w[p, g, e] = bf16(W^T[64g + (p-1)/2, e]) on odd p, 0.0 on even p.
  - PE then runs ONLY regular bf16 matmuls (64 accumulating MMs per
    256-token slab, moving N=256) - no PE transposes, no PSUM round-trip
    for x, and the dense MM stream keeps the HAM clock-gate warm.
  - All XBAR transposes are serialized on the sync queue (concurrent
    XBAR transposes hard-wedge the device); output DMAs interleave on the
    same queue two slabs behind, so nothing ever runs beside the XBAR.
  - out^T chunks DMA to DRAM as [64, 2048]; host transposes/concats (4MB).
"""
import numpy as np
import ml_dtypes

import concourse.mybir as mybir
from concourse import bacc
from concourse.tile import TileContext
from concourse.bass_utils import run_bass_kernel_spmd

TOKENS = 16384
HIDDEN = 4096
EXPERTS = 64
NCORES = 8
TSHARD = TOKENS // NCORES          # 2048 tokens per core
C = 2 * HIDDEN                     # 8192 u16 columns
G = C // 128                       # 64 column-groups per transpose
SLAB = 256                         # tokens per XBAR transpose
NSLAB = TSHARD // SLAB             # 8
FIXG = 16                          # g-groups per garbage-fix call

BF16 = mybir.dt.bfloat16
F32 = mybir.dt.float32

_cache = {}


def _build():
    if "nc" in _cache:
        return _cache["nc"]
    nc = bacc.Bacc("TRN2", target_bir_lowering=False, debug=False,
                   num_devices=NCORES)
    x_d = nc.dram_tensor("x", [TSHARD, C], BF16, kind="ExternalInput")
    w_d = nc.dram_tensor("w", [128, G * EXPERTS], BF16, kind="ExternalInput")
    m_d = nc.dram_tensor("m", [128, FIXG * SLAB], mybir.dt.uint16,
                         kind="ExternalInput")
    o_d = nc.dram_tensor("out", [EXPERTS, TSHARD], F32, kind="ExternalOutput")
    x = x_d.ap()
    o = o_d.ap()

    with TileContext(nc) as tc:
        with tc.tile_pool(name="consts", bufs=1) as cpool, \
             tc.tile_pool(name="xt", bufs=4) as xtpool, \
             tc.tile_pool(name="ps", bufs=2, space="PSUM") as pspool, \
             tc.tile_pool(name="ot", bufs=2) as otpool:
            w_sb = cpool.tile([128, G, EXPERTS], BF16)
            nc.sync.dma_start(out=w_sb, in_=w_d.ap())
            mask = cpool.tile([128, FIXG, SLAB], mybir.dt.uint16)
            nc.sync.dma_start(out=mask, in_=m_d.ap())

            outs = []

            def drain_out(upto):
                while len(outs) > upto:
                    s0, ot0 = outs.pop(0)
                    nc.sync.dma_start(
                        out=o[:, s0 * SLAB:(s0 + 1) * SLAB], in_=ot0)

            for s in range(NSLAB):
                xt = xtpool.tile([128, G, SLAB], BF16)
                nc.sync.dma_start_transpose(
                    xt, x[s * SLAB:(s + 1) * SLAB, :])
                # keep out-DMAs 2 slabs behind so they never stall the queue
                drain_out(2)

                po = pspool.tile([EXPERTS, SLAB], F32)
                for j in range(G // FIXG):
                    xv = xt[:, j * FIXG:(j + 1) * FIXG, :].bitcast(
                        mybir.dt.uint16)
                    nc.vector.tensor_tensor(
                        out=xv, in0=xv, in1=mask,
                        op=mybir.AluOpType.bitwise_and)
                    for g in range(j * FIXG, (j + 1) * FIXG):
                        nc.tensor.matmul(po, w_sb[:, g, :], xt[:, g, :],
                                         start=(g == 0), stop=(g == G - 1))
                ot = otpool.tile([EXPERTS, SLAB], F32)
                nc.vector.tensor_copy(ot, po)
                outs.append((s, ot))
            drain_out(0)

    nc.compile()
    _cache["nc"] = nc
    return nc


def _prep_w(weights_int8, scales):
    wt = (weights_int8.astype(np.float32)
          * scales.astype(np.float32)[:, None]).T   # [HIDDEN, EXPERTS]
    warr = np.zeros((128, G, EXPERTS), dtype=ml_dtypes.bfloat16)
    warr[1::2, :, :] = wt.reshape(G, 64, EXPERTS).transpose(1, 0, 2)
    return np.ascontiguousarray(warr.reshape(128, G * EXPERTS))


def _prep_mask():
    m = np.zeros((128, FIXG * SLAB), dtype=np.uint16)
    m[1::2, :] = 0xFFFF          # odd partitions hold real data - keep
    return m


def _make_inmaps(x, weights_int8, scales):
    warr = _prep_w(weights_int8, scales)
    marr = _prep_mask()
    xu = np.ascontiguousarray(x, dtype=np.float32).view(np.uint16)
    xv = xu.reshape(TOKENS, C).view(ml_dtypes.bfloat16)
    return [
        {"x": xv[c * TSHARD:(c + 1) * TSHARD], "w": warr, "m": marr}
        for c in range(NCORES)
    ]


def kernel(x, weights_int8, scales):
    nc = _build()
    in_maps = _make_inmaps(x, weights_int8, scales)
    res = run_bass_kernel_spmd(nc, in_maps, core_ids=list(range(NCORES)))
    out = np.concatenate(
        [np.asarray(res.results[c]["out"]).T for c in range(NCORES)], axis=0)
    return np.ascontiguousarray(out, dtype=np.float32)


# revision 12
# speedup vs baseline: 1.1143x; 1.0454x over previous
"""Int8RouterLinear TRN2 kernel: out[16384, 64] = x[16384, 4096] @ (W_int8 * scale)^T.

Strategy (data-parallel over 8 NeuronCores, 2048 tokens each):
  - x (f32) is viewed as pairs of u16; the hi-u16 of each f32 IS bf16(x)
    (truncated). The XBAR DMA-transpose (2-byte dtypes, DRAM->SBUF) loads
    x^T directly into SBUF: xt[p, g, t] = x_u16[t, g*128 + p]. Odd
    partitions hold bf16(x) for h = 64g + (p-1)/2; even partitions hold
    the lo-u16 garbage (can be NaN-patterned).
  - A DVE copy_predicated pass overwrites even (garbage) partitions with
    1.0 (mask doubles as data); the stationary weights have 0.0 there, so
    garbage contributes exactly 0 to the contraction.
  - The router weight is tiny: host dequantizes + lays it out as
    w[p, g, e] = bf16(W^T[64g + (p-1)/2, e]) on odd p, 0.0 on even p.
  - PE then runs ONLY regular bf16 matmuls (64 accumulating MMs per
    256-token slab, moving N=256) - no PE transposes, no PSUM round-trip
    for x, and the dense MM stream keeps the HAM clock-gate warm.
  - All XBAR transposes are serialized on the sync queue (concurrent
    XBAR transposes hard-wedge the device); output DMAs interleave on the
    same queue two slabs behind, so nothing ever runs beside the XBAR.
  - out^T chunks DMA to DRAM as [64, 2048]; host transposes/concats (4MB).
"""
import numpy as np
import ml_dtypes

import concourse.mybir as mybir
from concourse import bacc
from concourse.tile import TileContext
from concourse.bass_utils import run_bass_kernel_spmd

TOKENS = 16384
HIDDEN = 4096
EXPERTS = 64
NCORES = 8
TSHARD = TOKENS // NCORES          # 2048 tokens per core
C = 2 * HIDDEN                     # 8192 u16 columns
G = C // 128                       # 64 column-groups per transpose
SLAB = 512                         # tokens per XBAR transpose
NSLAB = TSHARD // SLAB             # 8
FIXG = 8                           # g-groups per garbage-fix call

BF16 = mybir.dt.bfloat16
F32 = mybir.dt.float32

_cache = {}


def _build():
    if "nc" in _cache:
        return _cache["nc"]
    nc = bacc.Bacc("TRN2", target_bir_lowering=False, debug=False,
                   num_devices=NCORES)
    x_d = nc.dram_tensor("x", [TSHARD, C], BF16, kind="ExternalInput")
    w_d = nc.dram_tensor("w", [128, G * EXPERTS], BF16, kind="ExternalInput")
    m_d = nc.dram_tensor("m", [128, FIXG * SLAB], mybir.dt.uint16,
                         kind="ExternalInput")
    o_d = nc.dram_tensor("out", [EXPERTS, TSHARD], F32, kind="ExternalOutput")
    x = x_d.ap()
    o = o_d.ap()

    with TileContext(nc) as tc:
        with tc.tile_pool(name="consts", bufs=1) as cpool, \
             tc.tile_pool(name="xt", bufs=2) as xtpool, \
             tc.tile_pool(name="ps", bufs=3, space="PSUM") as pspool, \
             tc.tile_pool(name="ot", bufs=2) as otpool:
            w_sb = cpool.tile([128, G, EXPERTS], BF16)
            nc.sync.dma_start(out=w_sb, in_=w_d.ap())
            mask = cpool.tile([128, FIXG, SLAB], mybir.dt.uint16)
            nc.sync.dma_start(out=mask, in_=m_d.ap())

            outs = []

            def drain_out(upto):
                while len(outs) > upto:
                    s0, ot0 = outs.pop(0)
                    nc.sync.dma_start(
                        out=o[:, s0 * SLAB:(s0 + 1) * SLAB], in_=ot0)

            for s in range(NSLAB):
                xt = xtpool.tile([128, G, SLAB], BF16)
                nc.sync.dma_start_transpose(
                    xt, x[s * SLAB:(s + 1) * SLAB, :])
                # keep out-DMAs 2 slabs behind so they never stall the queue
                drain_out(2)

                po = pspool.tile([EXPERTS, SLAB], F32)
                for j in range(G // FIXG):
                    xv = xt[:, j * FIXG:(j + 1) * FIXG, :].bitcast(
                        mybir.dt.uint16)
                    nc.vector.tensor_tensor(
                        out=xv, in0=xv, in1=mask,
                        op=mybir.AluOpType.bitwise_and)
                    for g in range(j * FIXG, (j + 1) * FIXG):
                        nc.tensor.matmul(po, w_sb[:, g, :], xt[:, g, :],
                                         start=(g == 0), stop=(g == G - 1))
                ot = otpool.tile([EXPERTS, SLAB], F32)
                nc.vector.tensor_copy(ot, po)
                outs.append((s, ot))
            drain_out(0)

    nc.compile()
    _cache["nc"] = nc
    return nc


def _prep_w(weights_int8, scales):
    wt = (weights_int8.astype(np.float32)
          * scales.astype(np.float32)[:, None]).T   # [HIDDEN, EXPERTS]
    warr = np.zeros((128, G, EXPERTS), dtype=ml_dtypes.bfloat16)
    warr[1::2, :, :] = wt.reshape(G, 64, EXPERTS).transpose(1, 0, 2)
    return np.ascontiguousarray(warr.reshape(128, G * EXPERTS))


def _prep_mask():
    m = np.zeros((128, FIXG * SLAB), dtype=np.uint16)
    m[1::2, :] = 0xFFFF          # odd partitions hold real data - keep
    return m


def _make_inmaps(x, weights_int8, scales):
    warr = _prep_w(weights_int8, scales)
    marr = _prep_mask()
    xu = np.ascontiguousarray(x, dtype=np.float32).view(np.uint16)
    xv = xu.reshape(TOKENS, C).view(ml_dtypes.bfloat16)
    return [
        {"x": xv[c * TSHARD:(c + 1) * TSHARD], "w": warr, "m": marr}
        for c in range(NCORES)
    ]


def kernel(x, weights_int8, scales):
    nc = _build()
    in_maps = _make_inmaps(x, weights_int8, scales)
    res = run_bass_kernel_spmd(nc, in_maps, core_ids=list(range(NCORES)))
    out = np.concatenate(
        [np.asarray(res.results[c]["out"]).T for c in range(NCORES)], axis=0)
    return np.ascontiguousarray(out, dtype=np.float32)
